# revision 23
# baseline (speedup 1.0000x reference)
"""MoE grouped-experts kernel for 8 Trainium2 NeuronCores.

Problem: T=4096 tokens, top-K=8, E=64 experts, D=2048, F=512, capacity C=768.
    y = combine(down(relu^2(up(dispatch(x)))), weights)

Sharding: expert-parallel, 8 expert slots per core. The host computes the
routing permutation, then load-balances: experts are sorted by routed-row
count and rank r goes to core r%8, slot-octile r//8, so slot s has the same
capacity cap[s] on every core (max count in its octile, rounded to 64).
This cuts matmul columns ~19% vs a single global capacity.

All DRAM tensors are laid out exactly like their SBUF tiles ([128, cols],
contraction dim on partitions), so every DMA is a contiguous [128, N] block
copy — few issues, large descriptors:
  xbt [128, 16*S]   bf16  dispatched tokens; slot s at col 16*off[s],
                          col dc*cap+c within a slot (d-chunk major)
  upw [8, 128, 8192] bf16 up-proj weights, col = dc*512 + f
  dnw [8, 128, 8192] bf16 down-proj weights, col = fb*2048 + d
  out [128, 16*S]   bf16  expert rows, transposed (d on partitions, slot
                          col dc*cap+c) — host un-transposes on combine

Device per slot: up-proj accumulates over 16 d-chunks into 4 PSUM banks
(one per f-block) so compute starts as soon as the first quarter of the
slot's DMA lands; relu^2 on vector. Down-proj keeps the dnw chunk
stationary and streams hT columns, so both GEMMs cost exactly 64*cap
PE-columns with no 128-tile quantization; PSUM evacuation alternates
scalar/vector; out DMA issued from gpsimd (SWDGE) to keep the scalar
engine free (final slot: chunked on scalar/HWDGE to shorten the drain
tail). A short dummy matmul burst at kernel start flips the PE HAM
throttle to full clock before real data arrives.

Host: final combine = gather rows by slot + weighted sum over K routes.
Duplicate (token, expert) routes are merged by summing combine weights.
"""

import numpy as np
import ml_dtypes

import concourse.bass as bass
import concourse.mybir as mybir
import concourse.tile as tile
from concourse import bass_utils

T, TOPK, E, D, F, C = 4096, 8, 64, 2048, 512, 768
NCORES = 8
SLOTS = E // NCORES        # expert slots per core
DT = D // 128              # 16 contraction chunks for up-proj
FB = F // 128              # 4 f-blocks / down-proj contraction chunks
GRAIN = 16                 # capacity rounding
WARM_MM = 6                # dummy matmuls to warm the PE clock gate

BF16 = mybir.dt.bfloat16
F32 = mybir.dt.float32

# Set by test harness to collect an NTFF profile; kernel() stores the
# BassKernelResults of the last run here either way.
TRACE = False
LAST_RESULTS = None


def _build_nc(caps):
    cap_max = max(caps)
    S = sum(caps)
    off = np.concatenate([[0], np.cumsum(caps)]).astype(int)

    nc = bass.Bass("TRN2")
    xbt = nc.dram_tensor("xbt", [128, DT * S], BF16, kind="ExternalInput")
    upw = nc.dram_tensor("upw", [SLOTS, 128, DT * F], BF16, kind="ExternalInput")
    dnw = nc.dram_tensor("dnw", [SLOTS, 128, FB * D], BF16, kind="ExternalInput")
    out = nc.dram_tensor("out", [128, DT * S], BF16, kind="ExternalOutput")

    copy = mybir.ActivationFunctionType.Copy
    mult = mybir.AluOpType.mult

    with tile.TileContext(nc) as tc:
        with (
            tc.tile_pool(name="xbtp", bufs=3) as xbt_pool,
            tc.tile_pool(name="upwp", bufs=3) as upw_pool,
            tc.tile_pool(name="dnwp", bufs=2) as dnw_pool,
            tc.tile_pool(name="htp", bufs=2 * FB) as ht_pool,
            tc.tile_pool(name="rlp", bufs=4) as rl_pool,
            tc.tile_pool(name="otp", bufs=2) as ot_pool,
            tc.tile_pool(name="wrm", bufs=1) as warm_pool,
            tc.tile_pool(name="psu", bufs=4, space="PSUM") as psu_pool,
            tc.tile_pool(name="psd", bufs=4, space="PSUM") as psd_pool,
        ):
            # PE warmup: the HAM clock gate needs ~3.4us of sustained matmul
            # activity to lift the PE from 1.2 to 2.4 GHz; burn that while
            # the first slot's DMA streams in. Inputs are junk, output is
            # never read.
            wt_w = warm_pool.tile([128, 128], BF16, tag="ww")
            wt_x = warm_pool.tile([128, 512], BF16, tag="wx")
            # gpsimd's preamble finishes earliest, so memsets there unblock
            # the warmup burst ~1us sooner than on vector
            nc.gpsimd.memset(wt_w[:], 0.0)
            nc.gpsimd.memset(wt_x[:], 0.0)
            wps = psd_pool.tile([128, 512], F32, tag="psd")
            for i in range(WARM_MM):
                nc.tensor.matmul(
                    wps[:], wt_w[:], wt_x[:],
                    start=(i == 0), stop=(i == WARM_MM - 1),
                )

            for s, cap in enumerate(caps):
                xt = xbt_pool.tile([128, DT * cap_max], BF16, tag="xbt")
                ut = upw_pool.tile([128, DT * F], BF16, tag="upw")
                dt_ = dnw_pool.tile([128, FB * D], BF16, tag="dnw")
                xbase = DT * off[s]
                # interleave x/weight chunks so the up-proj (which consumes
                # d-chunk g as soon as chunk g lands) starts early; finer
                # first chunks on slot 0 shorten the cold-start fill
                chunks = [(0, 2), (2, 2), (4, 4), (8, 4), (12, 4)] if s == 0 \
                    else [(0, 4), (4, 4), (8, 4), (12, 4)]
                # slot 0 is DMA-fill-bound: put its weight streams on the
                # scalar HWDGE ring so they issue in parallel with xbt on
                # the sync ring (the scalar ring is idle until ~20us)
                weng = nc.scalar if s == 0 else nc.sync
                for g0, gw in chunks:
                    weng.dma_start(
                        ut[:, g0 * 512:(g0 + gw) * 512],
                        upw[s, :, g0 * 512:(g0 + gw) * 512],
                    )
                    nc.sync.dma_start(
                        xt[:, g0 * cap:(g0 + gw) * cap],
                        xbt[:, xbase + g0 * cap: xbase + (g0 + gw) * cap],
                    )
                for g in range(2):
                    weng.dma_start(
                        dt_[:, g * 4096:(g + 1) * 4096],
                        dnw[s, :, g * 4096:(g + 1) * 4096],
                    )

                # up-proj: hT[f, c] = sum_d upw[d, f] * xbt[d, c], one PSUM
                # bank per f-block, accumulating across d-chunks
                hts = [
                    ht_pool.tile([128, cap_max], BF16, tag="ht", name=f"ht_{s}_{f}")
                    for f in range(FB)
                ]
                if cap <= 512:
                    spans = [(0, cap)]
                else:
                    spans = [(0, cap // 2), (cap // 2, cap - cap // 2)]
                for sp_off, sp_w in spans:
                    pss = [
                        psu_pool.tile([128, 512], F32, tag="psu",
                                      name=f"ps_{s}_{sp_off}_{fb}")
                        for fb in range(FB)
                    ]
                    for dc in range(DT):
                        for fb in range(FB):
                            nc.tensor.matmul(
                                pss[fb][:, :sp_w],
                                ut[:, dc * F + fb * 128: dc * F + (fb + 1) * 128],
                                xt[:, dc * cap + sp_off: dc * cap + sp_off + sp_w],
                                start=(dc == 0),
                                stop=(dc == DT - 1),
                            )
                    for fb in range(FB):
                        rl = rl_pool.tile([128, 512], BF16, tag="rl")
                        nc.vector.tensor_scalar_max(rl[:, :sp_w], pss[fb][:, :sp_w], 0.0)
                        nc.vector.tensor_tensor(
                            hts[fb][:, sp_off:sp_off + sp_w],
                            rl[:, :sp_w], rl[:, :sp_w], mult,
                        )

                # down-proj, dnw stationary: outT[d, c] = sum_f dnw[f, d] * hT[f, c]
                obase = DT * off[s]
                ot = ot_pool.tile([128, DT * cap_max], BF16, tag="ot")
                last_slot = (s == len(caps) - 1)
                evac_i = 0
                for dc in range(DT):
                    tail_dc = last_slot and dc >= DT - 4
                    for sp_i, (sp_off, sp_w) in enumerate(spans):
                        ps2 = psd_pool.tile([128, 512], F32, tag="psd")
                        for fb in range(FB):
                            nc.tensor.matmul(
                                ps2[:, :sp_w],
                                dt_[:, fb * D + dc * 128: fb * D + (dc + 1) * 128],
                                hts[fb][:, sp_off:sp_off + sp_w],
                                start=(fb == 0),
                                stop=(fb == FB - 1),
                            )
                        dst = ot[:, dc * cap + sp_off: dc * cap + sp_off + sp_w]
                        if evac_i % 2 == 0:
                            nc.scalar.activation(dst, ps2[:, :sp_w], copy)
                        else:
                            nc.vector.tensor_copy(dst, ps2[:, :sp_w])
                        if tail_dc:
                            # drain per span so the very last transfer (and
                            # its completion receipt) is tiny
                            nc.scalar.dma_start(
                                out[:, obase + dc * cap + sp_off:
                                    obase + dc * cap + sp_off + sp_w],
                                dst,
                            )
                        evac_i += 1
                    # drain the final slot's head in quarters on the HWDGE
                    # ring so its data is long gone before the tail
                    if last_slot and dc < DT - 4 and dc % 4 == 3:
                        g0 = (dc - 3) * cap
                        nc.scalar.dma_start(
                            out[:, obase + g0: obase + (dc + 1) * cap],
                            ot[:, g0: (dc + 1) * cap],
                        )
                if not last_slot:
                    nc.gpsimd.dma_start(
                        out[:, obase: obase + DT * cap], ot[:, :DT * cap]
                    )
    _legalize_waits(nc)
    return nc


def _legalize_waits(nc):
    """Walrus codegen accepts only 1 sync wait per instruction (2 on
    EventSemaphore). Tile's scheduler sometimes attaches more (slot-reuse +
    queue-capacity + data deps). Split the excess onto same-engine
    EventSemaphore instructions inserted immediately before the offender —
    the sequencer executes them in program order, so semantics are
    unchanged."""
    import bass_rust
    n_new = 0
    for fn in nc.m.functions:
        for blk in fn.blocks:
            insts = blk.instructions  # live list
            i = 0
            while i < len(insts):
                inst = insts[i]
                si = inst.sync_info
                nw = len(si.on_wait) if si is not None else 0
                if isinstance(inst, mybir.InstEventSemaphore) or nw <= 1:
                    i += 1
                    continue
                waits = list(si.on_wait)
                # keep the DMA-queue wait inline if present, else the last one
                keep_i = len(waits) - 1
                for j, w in enumerate(waits):
                    if w.ant_name.startswith(("DMAHW", "DMASW")):
                        keep_i = j
                        break
                keep = [waits[keep_i]]
                move = [w for j, w in enumerate(waits) if j != keep_i]
                inst.sync_info = bass_rust.SyncInfo(
                    on_wait=keep, on_update=list(si.on_update)
                )
                for k in range(0, len(move), 2):
                    ev = mybir.InstEventSemaphore(
                        name=f"I-lgl-{n_new}", ins=[], outs=[], engine=inst.engine
                    )
                    ev.sync_info = bass_rust.SyncInfo(
                        on_wait=move[k:k + 2], on_update=[]
                    )
                    insts.insert(i, ev)
                    n_new += 1
                    i += 1
                i += 1
    return n_new


_NC_CACHE = {}


def _routing(indices, weights):
    """Merged routing tables. Returns (ge, gtok, gkeep, grp arrays, per-
    expert clipped counts, and the per-(t,k) combine weights)."""
    N = T * TOPK
    flat_e = indices.reshape(-1)
    order = np.argsort(flat_e, kind="stable")
    sorted_e = flat_e[order]
    tok = (order // TOPK).astype(np.int32)
    counts = np.bincount(flat_e, minlength=E)
    starts = np.cumsum(counts) - counts
    pos = np.arange(N) - starts[sorted_e]
    valid = pos < C
    # merge duplicate (token, expert) routes: the dispatched row is
    # identical, so they share a slot and their weights sum on combine
    same = (np.diff(sorted_e) == 0) & (np.diff(tok) == 0)
    is_start = np.concatenate([[True], ~same])
    grp = np.cumsum(is_start) - 1
    rep_idx = np.flatnonzero(is_start)
    ge = sorted_e[rep_idx]
    gtok = tok[rep_idx]
    gcounts = np.bincount(ge, minlength=E)
    gstarts = np.cumsum(gcounts) - gcounts
    gpos = np.arange(len(rep_idx)) - gstarts[ge]
    gkeep = gpos < C
    gc = np.minimum(gcounts, C)
    # per-(t, k) combine weight, zeroed for capacity-overflow slots
    wv = np.zeros(N, np.float32)
    wv[order] = weights.reshape(-1)[order] * valid
    return ge, gtok, gkeep, grp, gpos, order, gc, wv.reshape(T, TOPK)


def _plan(gc):
    """Load-balanced expert->(core, slot) assignment with per-slot
    capacities. Rank r (by descending count) -> core r%8, octile r//8;
    octiles map to slots in ascending-capacity order so the pipeline
    starts on the cheapest slot."""
    rank = np.argsort(-gc, kind="stable")
    caps_oct = []
    for o in range(SLOTS):
        mx = int(gc[rank[NCORES * o]])
        caps_oct.append(max(GRAIN, -(-mx // GRAIN) * GRAIN))
    # ascending capacity, but keep the largest slot mid-pipeline: the
    # first slot paces startup (cheap = good) and the last slot paces the
    # drain tail (also cheap = good)
    asc = list(range(SLOTS - 1, -1, -1))  # octiles in ascending-cap order
    order_slots = asc[:SLOTS // 2] + [asc[-1]] + asc[SLOTS // 2:-1]
    oct_to_slot = {o: s for s, o in enumerate(order_slots)}
    caps = [0] * SLOTS
    core_of = np.zeros(E, np.int64)
    slot_of = np.zeros(E, np.int64)
    for r, e in enumerate(rank):
        o, m = r // NCORES, r % NCORES
        s = oct_to_slot[o]
        caps[s] = caps_oct[o]
        core_of[e] = m
        slot_of[e] = s
    return caps, core_of, slot_of


def kernel(x, weights, indices, up_w, down_w):
    global _NC_CACHE, LAST_RESULTS
    bf16 = ml_dtypes.bfloat16

    ge, gtok, gkeep, grp, gpos, order, gc, wv = _routing(indices, weights)
    caps, core_of, slot_of = _plan(gc)
    S = sum(caps)
    off = np.concatenate([[0], np.cumsum(caps)]).astype(int)
    x_bf = x.astype(bf16)

    # expert id per (core, slot)
    e_at = np.zeros((NCORES, SLOTS), np.int64)
    e_at[core_of, slot_of] = np.arange(E)

    in_maps = []
    for m in range(NCORES):
        xbt = np.zeros((128, DT * S), bf16)
        upw = np.empty((SLOTS, 128, DT * F), bf16)
        dnw = np.empty((SLOTS, 128, FB * D), bf16)
        for s in range(SLOTS):
            e = e_at[m, s]
            cap = caps[s]
            sel = gtok[(ge == e) & gkeep]
            # [cnt, D] -> [D, cnt] -> [16, 128, cnt] -> [128, 16, cnt]
            a = np.ascontiguousarray(x_bf[sel].T).reshape(DT, 128, len(sel))
            xv = xbt[:, DT * off[s]: DT * off[s + 1]].reshape(128, DT, cap)
            xv[:, :, :len(sel)] = a.transpose(1, 0, 2)
            upw[s] = (
                up_w[e].reshape(DT, 128, F).transpose(1, 0, 2).reshape(128, DT * F)
            ).astype(bf16)
            dnw[s] = (
                down_w[e].reshape(FB, 128, D).transpose(1, 0, 2).reshape(128, FB * D)
            ).astype(bf16)
        in_maps.append({"xbt": xbt, "upw": upw, "dnw": dnw})

    key = tuple(caps)
    if key not in _NC_CACHE:
        _NC_CACHE[key] = _build_nc(caps)
    nc = _NC_CACHE[key]

    res = bass_utils.run_bass_kernel_spmd(
        nc, in_maps, core_ids=list(range(NCORES)), trace=TRACE
    )
    LAST_RESULTS = res

    # un-transpose: out[p, 16*off[s] + dc*cap + c] = ob[c, dc*128 + p]
    segs = []
    for r in res.results:
        o = r["out"]
        for s in range(SLOTS):
            cap = caps[s]
            seg = o[:, DT * off[s]: DT * off[s + 1]].reshape(128, DT, cap)
            segs.append(seg.transpose(2, 1, 0).reshape(cap, D))
    segs.append(np.zeros((1, D), res.results[0]["out"].dtype))
    rows = np.concatenate(segs)
    # global row of expert e's slot p: core*S + off[slot] + p
    base_e = core_of * S + off[slot_of]
    N = T * TOPK
    inv = np.full(N, NCORES * S, np.int64)  # sentinel: appended zero row
    keep_slot = gkeep[grp]
    inv[order[keep_slot]] = base_e[ge[grp[keep_slot]]] + gpos[grp[keep_slot]]
    inv = inv.reshape(T, TOPK)

    y = np.zeros((T, D), np.float32)
    for k in range(TOPK):
        y += rows[inv[:, k]].astype(np.float32) * wv[:, k, None]
    return y
